# revision 69
# baseline (speedup 1.0000x reference)
"""AttentionPairBias distributed Trainium2 kernel (v2).

Sharding: 1024 query rows split across 8 cores (128 each); z_ij sharded over
i and host-permuted to [c_z, ij] quarter-interleaved chunks.  SPMD via token
rotation (softmax/attention invariant to j-permutation).

v2 layout of the device program (no inter-phase barriers; Tile deps only):
  - z stream (phase C) mostly in fp8e4m3: chunk tile zz[128, 2, 2048]
    holds z and z^2 side by side; ONE DoubleRow matmul per 512-col quarter
    contracts K=256 over (z, z^2) against block-shifted (64*wb'', 64*e16/
    e17) weights producing 16 head rows + sum-z + sum-z^2 in a single PE
    pass at 0.5 cyc/col.  Every BFEVth chunk streams bf16 instead (square
    on DVE at 2x; two plain matmul passes) to balance PE vs the
    elementwise engines.  Squares rotate Pool/Pool/DVE.  Stats go to a
    DRAM braw[128, 32768] bf16 via batched stage tiles; the j-transposing
    gathers read it back as [i, j] planes.
  - AdaLN + QKV (phase B) in bf16 runs concurrently, interleaved into the
    chunk loop so each engine's in-order stream makes progress on both.
  - The z stream is split even-j-first/odd-j-second: after the even half
    lands, each head's half-0 logits+bias+exp runs overlapped with the
    odd-half stream (denominators accumulate per head); the tail does
    half-1, folds both denominators, and runs transpose+AV.
  - rs = exp(-0.5*ln(var+eps) - ln 64) keeps ACT on the natural_log_exp
    table straight into phase E's Exp (no act-table thrash); the 1/64
    undoes the x64 scaling that keeps the fp8 pair-bias weights normal.
  - Attention (phase E, v3): logits are produced TRANSPOSED ([j, i]) --
    per 128-j block, a kT-weighted QK matmul plus an X(=bias*rs)-as-
    weights transpose accumulate into one [128, 4x128i] psum bank; Exp
    then emits E^T directly, so AV consumes it with no PE transposes,
    no psum->sbuf A copies, and no per-head normalize.  V carries 1.0
    in its 8 pad columns per head (added in-psum via a one-hot vpad
    matmul step), so each AV accumulation yields the softmax
    denominator rows for free; og = (ops0+ops1)*gT*rexp where rexp
    broadcasts 1/den from the pad rows via a one-hot selector matmul.
    Half-0 AV runs inside the odd-z overlap window into its own psum
    bank (accumulation groups must close within a phase); half-1 bias
    rows are prefetched into the dead bh0 slots before the head loop --
    pooled per-head gathers would put a full DMA round-trip on the
    cadence.  Dummy PE matmuls bridge the rs1 stall to hold the
    p-state ramp.
"""

import os
import sys

import numpy as np

for _p in ("/opt/trn_rl_repo",):
    if _p not in sys.path and os.path.isdir(_p):
        sys.path.insert(0, _p)

import concourse.bass as bass
import concourse.tile as tile
from concourse import mybir
from concourse.bass_utils import run_bass_kernel_spmd
from concourse.masks import make_identity

# ---------------------------------------------------------------------------
# The walrus build in this container accepts at most ONE sync-wait command per
# instruction, while current Tile emits multi-wait sync_info.  Patch the BIR
# just before compilation: extra waits move onto preceding same-engine NoOps
# (sequencer executes them in order, so semantics are identical).
import json as _json

import concourse.bass_utils as _bass_utils
import concourse.bass2jax as _bass2jax

_ORIG_COMPILE_BIR = _bass_utils.compile_bir_kernel


def _split_sync_waits(bir_json, max_waits=1):
    d = _json.loads(bir_json)
    ctr = 0
    for fn in d["functions"]:
        for bb in fn["blocks"]:
            new = []
            for inst in bb["instructions"]:
                si = inst.get("sync_info")
                if si and si.get("on_wait") and len(si["on_wait"]) > max_waits:
                    waits = si["on_wait"]
                    extra, keep = waits[:-max_waits], waits[-max_waits:]
                    for w in extra:
                        ctr += 1
                        new.append({
                            "debug": inst.get("debug", 0),
                            "engine": inst["engine"],
                            "ins": [], "outs": [],
                            "name": f"WSPL-{ctr}",
                            "opcode": "NoOp",
                            "sync_info": {"on_update": [], "on_wait": [w]},
                        })
                    si["on_wait"] = keep
                new.append(inst)
            bb["instructions"] = new
    return _json.dumps(d).encode()


def _patched_compile_bir_kernel(bir_json, tmpdir, neff_name="file.neff"):
    return _ORIG_COMPILE_BIR(_split_sync_waits(bir_json), tmpdir, neff_name)


_bass_utils.compile_bir_kernel = _patched_compile_bir_kernel
_bass2jax.compile_bir_kernel = _patched_compile_bir_kernel
# ---------------------------------------------------------------------------

B, C_S, C_Z, H, D = 1, 384, 128, 16, 24
N = 1024
NCORES = 8
DP = 32                   # padded head dim
HDP = H * DP              # 512
KC = C_S // 128           # 3 contraction chunks over c_s
MC4 = HDP // 128          # 4 chunks over padded heads
ZCH = 2048                # ij columns per z chunk
EPS = 1e-5
SCALE = 1.0 / float(np.sqrt(np.float32(D)))
WBSCALE = 64.0            # fp8 weight scale; undone via the rs exp bias

IB = N // NCORES          # 128 query rows per core
IJ = IB * N               # ij index space per core
NZC = IJ // ZCH           # 64 z chunks
NJC = N // 128            # 8 j chunks
IJ4 = IJ // 4             # within-quarter ij space

f32 = mybir.dt.float32
bf16 = mybir.dt.bfloat16
f8 = mybir.dt.float8e4
AF = mybir.ActivationFunctionType
ALU = mybir.AluOpType

USE_Z8 = os.environ.get("KERNEL_Z8", "1") == "1"   # fp8 DoubleRow z path
ZDT = f8 if USE_Z8 else bf16
# every BFEVth chunk streams in bf16 (square on DVE at 2x; 2-pass matmul):
# balances PE vs DVE/ACT/Pool.  0 disables mixing.
BFEV = int(os.environ.get("KERNEL_BFEV", "13")) if USE_Z8 else 0


def _chunk_is_bf16(t):
    return BFEV > 0 and t % BFEV == BFEV - 1


NZB = sum(1 for t in range(NZC) if _chunk_is_bf16(t))   # bf16 chunks
NZ8 = NZC - NZB
# position of chunk t within its dtype-ordered stream (host packs ascending)
I8POS = {t: k for k, t in enumerate(t for t in range(NZC) if not _chunk_is_bf16(t))}
IBPOS = {t: k for k, t in enumerate(t for t in range(NZC) if _chunk_is_bf16(t))}

_CACHED = {}


def _build_program():
    nc = bass.Bass()
    p = {}
    decl = [
        ("z_t", [C_Z, max(NZ8, 1) * ZCH], ZDT),
        ("z_tb", [C_Z, max(NZB, 1) * ZCH], bf16),
        ("a_in", [N, C_S], f32), ("s_in", [N, C_S], f32),
        ("w_ws", [C_S, C_S], bf16), ("w_wns", [C_S, C_S], bf16),
        ("b_s", [C_S], f32),
        ("w_q", [C_S, HDP], bf16), ("b_q", [HDP], f32),
        ("w_k", [C_S, HDP], bf16), ("w_v", [C_S, HDP], bf16),
        ("w_g", [C_S, HDP], bf16),
        ("wb2", [C_Z, 4, 2, 128] if USE_Z8 else [C_Z, 2, 32], ZDT),
        ("wb2b", [C_Z, 2, 32], bf16),
        ("w_o", [HDP, C_S], bf16), ("w_sg", [C_S, C_S], bf16),
        ("b_sg", [C_S], f32), ("w4exp", [128, 128], f32),
        ("vpad", [1, HDP], bf16),
    ]
    for name, shape, dt_ in decl:
        p[name] = nc.declare_dram_parameter(name, shape, dt_, isOutput=False)
    p["out"] = nc.declare_dram_parameter("out", [IB, C_S], f32, isOutput=True)

    with tile.TileContext(nc) as tc:
        _emit(tc, p)
    return nc


def _emit(tc, p):
    from contextlib import ExitStack

    nc = tc.nc
    ctx = ExitStack()
    with ctx:
        singles = ctx.enter_context(tc.tile_pool(name="singles", bufs=1))
        persist = ctx.enter_context(tc.tile_pool(name="persist", bufs=1))
        _ps3 = os.environ.get("KERNEL_PSS3", "1") == "1"
        ps_stat = ctx.enter_context(tc.tile_pool(name="ps_stat", bufs=2 if os.environ.get("KERNEL_BIG3", "0") == "1" else (3 if _ps3 else 2),
                                                 space="PSUM"))
        _bg3 = os.environ.get("KERNEL_BIG3", "0") == "1"
        ps_big = ctx.enter_context(tc.tile_pool(name="ps_big", bufs=3 if _bg3 else 2,
                                                space="PSUM"))
        _smd = os.environ.get("KERNEL_SMDON", "0") == "1"
        ps_sm = ctx.enter_context(tc.tile_pool(name="ps_sm",
                                               bufs=int(os.environ.get("KERNEL_SMB", "1")),
                                               space="PSUM"))
        ps_o = ctx.enter_context(tc.tile_pool(name="ps_o", bufs=1,
                                              space="PSUM"))

        dma_z = nc.sync.dma_start        # SP queue: z stream + gathers + out
        dma_w = nc.scalar.dma_start      # ACT queue: weights + activations
        dma_p = nc.sync.dma_start        # (SWDGE desc-gen holds the Pool
        # engine ~1us per DMA in the cost model; keep Pool queue empty)

        ident = singles.tile([128, 128], bf16, name="ident", tag="ident")
        make_identity(nc, ident)
        ident_f = singles.tile([128, 128], f32, name="ident_f", tag="ident_f")
        make_identity(nc, ident_f)
        eps_t = singles.tile([128, 1], f32, name="eps_t", tag="eps")
        nc.vector.memset(eps_t, EPS)
        nlb_t = singles.tile([128, 1], f32, name="nlb_t", tag="nlb")
        nc.vector.memset(nlb_t, -float(np.log(WBSCALE)))
        ones1 = singles.tile([1, 128], bf16, name="ones1", tag="ones1")
        nc.vector.memset(ones1, 1.0)
        vpad_sb = singles.tile([1, HDP], bf16, name="vpad_sb", tag="vpad_sb")
        dma_w(out=vpad_sb, in_=p["vpad"][:, :])

        # ---- weights to SBUF (ACT queue) ----
        def wload(name, ap, shape, dt_=bf16):
            w = singles.tile(shape, dt_, name=name, tag=name)
            dma_w(out=w, in_=ap)
            return w

        r3 = "(kc pp) o -> pp kc o"
        # load order = need order: z-projection weights first, F-stage last
        if USE_Z8:
            wb2_sb = wload("wb2_sb", p["wb2"][:, :, :, :], [C_Z, 4, 2, 128], ZDT)
        else:
            wb2_sb = wload("wb2_sb", p["wb2"][:, :, :], [C_Z, 2, 32], ZDT)
        wb2b_sb = wload("wb2b_sb", p["wb2b"][:, :, :], [C_Z, 2, 32], bf16)
        ws_sb = wload("ws_sb", p["w_ws"][:, :].rearrange(r3, pp=128), [128, KC, C_S])
        wns_sb = wload("wns_sb", p["w_wns"][:, :].rearrange(r3, pp=128), [128, KC, C_S])
        bs_sb = wload("bs_sb", p["b_s"][:].rearrange("(mc pp) -> pp mc", pp=128),
                      [128, KC], f32)
        wk_sb = wload("wk_sb", p["w_k"][:, :].rearrange(r3, pp=128), [128, KC, HDP])
        wq_sb = wload("wq_sb", p["w_q"][:, :].rearrange(r3, pp=128), [128, KC, HDP])
        wg_sb = wload("wg_sb", p["w_g"][:, :].rearrange(r3, pp=128), [128, KC, HDP])
        wv_sb = wload("wv_sb", p["w_v"][:, :].rearrange(r3, pp=128), [128, KC, HDP])
        # tail-only weights load later (keeps early ACT SEQ free)
        wsg_sb = singles.tile([128, KC, C_S], bf16, name="wsg_sb", tag="wsg_sb")
        wo_sb = singles.tile([128, MC4, C_S], bf16, name="wo_sb", tag="wo_sb")
        bq_sb = singles.tile([128, MC4], f32, name="bq_sb", tag="bq_sb")
        w4_sb = singles.tile([128, 128], f32, name="w4_sb", tag="w4_sb")
        bsg_bc = singles.tile([128, C_S], f32, name="bsg_bc", tag="bsg_bc")

        def late_weights():
            dma_w(out=wsg_sb, in_=p["w_sg"][:, :].rearrange(r3, pp=128))
            dma_w(out=wo_sb, in_=p["w_o"][:, :].rearrange(r3, pp=128))
            dma_w(out=bq_sb, in_=p["b_q"][:].rearrange("(mc pp) -> pp mc", pp=128))
            dma_w(out=w4_sb, in_=p["w4exp"][:, :])
            bsg_ap = p["b_sg"][:]
            dma_w(
                out=bsg_bc,
                in_=bass.AP(tensor=bsg_ap.tensor, offset=bsg_ap.offset,
                            ap=[[0, 128]] + [list(d) for d in bsg_ap.ap]),
            )

        # ---- persistent activations ----
        dram = ctx.enter_context(tc.tile_pool(name="dram", bufs=1, space="DRAM"))
        # separate even-j / odd-j stats tensors: half-0 gathers then depend
        # only on the even-chunk stage writes, unlocking the C/E overlap
        IJ4H = IJ4 // 2
        braw_h = [dram.tile([128, IJ4H], bf16, name=f"braw{j}", tag=f"braw{j}")
                  for j in range(2)]
        kT = [persist.tile([128, N], bf16, name=f"kT{m}", tag=f"kT{m}") for m in range(MC4)]
        V = [persist.tile([128, HDP], bf16, name=f"V{j}", tag=f"V{j}") for j in range(NJC)]
        qT = [persist.tile([128, IB], bf16, name=f"qT{m}", tag=f"qT{m}") for m in range(MC4)]
        gT = [persist.tile([128, IB], bf16, name=f"gT{m}", tag=f"gT{m}") for m in range(MC4)]
        siT3 = persist.tile([128, KC, IB], bf16, name="siT3", tag="siT3")
        sg = persist.tile([IB, C_S], f32, name="sg", tag="sg")

        # =================================================================
        # Phase B emission, chopped into closures so it can be interleaved
        # with the z-chunk loop (engines execute their streams in order).
        # =================================================================
        # created before the scoped B/C pools: these outlive them (LIFO pools)
        p2 = ctx.enter_context(tc.tile_pool(name="p2", bufs=1))
        att = ctx.enter_context(tc.tile_pool(name="att", bufs=int(os.environ.get("KERNEL_ATTB", "2"))))

        bc_ctx = ExitStack()
        adb = bc_ctx.enter_context(tc.tile_pool(name="adbuf", bufs=1))
        ad = bc_ctx.enter_context(tc.tile_pool(name="adaln", bufs=3))
        sT3 = adb.tile([128, KC, N], bf16, name="sT3", tag="sT3")
        lnaT3 = adb.tile([128, KC, N], bf16, name="lnaT3", tag="lnaT3")
        aT3 = adb.tile([128, KC, N], bf16, name="aT3", tag="aT3")
        mvs = adb.tile([128, 32], f32, name="mvs", tag="mvs")
        rstd = adb.tile([128, 16], f32, name="rstd", tag="rstd")
        # 16 raw input tiles stay live until rstd is known; freed before the
        # odd-chunk window.  Allocated after the z pools (stack order).
        xs = []

        b_tasks = []
        _cp = [0]

        _cppat = os.environ.get("KERNEL_CPPAT", "DA")

        def copy2(out, in_):
            # rotate psum->sbuf copies DVE/ACT (GPSIMD can't touch PSUM)
            _cp[0] += 1
            if _cppat[_cp[0] % len(_cppat)] == "D":
                nc.vector.tensor_copy(out=out, in_=in_)
            else:
                nc.scalar.activation(out=out, in_=in_, func=AF.Copy)

        def psmf():
            return ps_sm.tile([128, 512], f32, name="pss", tag="pss")

        def psmb():
            # bf16 view of an f32 small-psum slot (pools bill whole banks)
            return psmf().bitcast(bf16)[:, 0:512]

        def ln_stats(src, xi, mvcol, keep_raw=False):
            def run():
                x = xs[xi]
                dma_w(out=x, in_=src)
                st = ad.tile([128, nc.vector.BN_STATS_DIM], f32, name="ln_st", tag="ln_st")
                nc.vector.bn_stats(out=st, in_=x)
                nc.vector.bn_aggr(out=mvs[:, mvcol:mvcol + 2], in_=st)
                if keep_raw:
                    pt = psmf()
                    for k in range(KC):
                        nc.tensor.transpose(pt[:, k * 128:(k + 1) * 128],
                                            x[:, k * 128:(k + 1) * 128], ident_f)
                    copy2(siT3[:, :, :], pt[:, 0:C_S].rearrange("p (k c) -> p k c", c=IB))
            return run

        def ln_norm(xi, dstT3, tt, mvcol):
            def run():
                y = ad.tile([128, C_S], bf16, name="ln_y", tag="ln_y")
                nc.gpsimd.tensor_scalar(out=y, in0=xs[xi], scalar1=mvs[:, mvcol:mvcol + 1],
                                        scalar2=rstd[:, mvcol // 2:mvcol // 2 + 1],
                                        op0=ALU.subtract, op1=ALU.mult)
                pt = psmb()
                for k in range(KC):
                    nc.tensor.transpose(pt[:, k * 128:(k + 1) * 128],
                                        y[:, k * 128:(k + 1) * 128], ident)
                copy2(dstT3[:, :, tt * 128:(tt + 1) * 128],
                      pt[:, 0:C_S].rearrange("p (k c) -> p k c", c=128))
            return run

        def rstd_batch(lo):
            # rstd[lo/2:lo/2+8] = exp(-0.5*ln(var+eps)); vars at mvs[:,lo+1::2]
            def run():
                lnv = ad.tile([128, 8], f32, name="lnv", tag="lnv")
                var_view = bass.AP(tensor=mvs.tensor, offset=mvs.offset + lo + 1,
                                   ap=[list(mvs.ap[0]), [2, 8]])
                nc.scalar.activation(out=lnv, in_=var_view, func=AF.Ln, bias=eps_t)
                nc.scalar.activation(out=rstd[:, lo // 2:lo // 2 + 8], in_=lnv,
                                     func=AF.Exp, scale=-0.5)
            return run

        # s-tiles then a-tiles through the same 8 x buffers
        for tt in range(N // 128):
            b_tasks.append(ln_stats(p["s_in"][tt * 128:(tt + 1) * 128, :], tt,
                                    2 * tt, keep_raw=(tt == 0)))
        b_tasks.append(rstd_batch(0))
        for tt in range(N // 128):
            b_tasks.append(ln_norm(tt, sT3, tt, 2 * tt))
        b_tasks.append(late_weights)
        for tt in range(N // 128):
            b_tasks.append(ln_stats(p["a_in"][tt * 128:(tt + 1) * 128, :],
                                    (8 + tt) % int(os.environ.get("KERNEL_XB", "16")),
                                    16 + 2 * tt))
        b_tasks.append(rstd_batch(16))
        for tt in range(N // 128):
            b_tasks.append(ln_norm((8 + tt) % int(os.environ.get("KERNEL_XB", "16")),
                                   lnaT3, tt, 16 + 2 * tt))

        # final-gate sigmoid (raw s_i), emitted inside the sigmoid block
        def sg_task():
            psg = ps_big.tile([128, 512], f32, name="psb", tag="psb")
            for k in range(KC):
                nc.tensor.matmul(psg[0:IB, 0:C_S], lhsT=siT3[:, k, :], rhs=wsg_sb[:, k, :],
                                 start=(k == 0), stop=(k == KC - 1))
            sgl = ad.tile([IB, C_S], f32, name="sgl", tag="sgl")
            nc.vector.tensor_add(out=sgl, in0=psg[0:IB, 0:C_S], in1=bsg_bc)
            nc.scalar.activation(out=sg, in_=sgl, func=AF.Sigmoid)
        b_tasks.append(sg_task)

        # adaln chains: a = sigmoid(s@ws + bs) * ln(a) + s@wns  (transposed)
        def adaln_task(m, half):
            def run():
                sl = slice(half * 512, (half + 1) * 512)
                p1 = ps_big.tile([128, 512], f32, name="psb", tag="psb")
                for k in range(KC):
                    nc.tensor.matmul(p1[:, 0:512], lhsT=ws_sb[:, k, m * 128:(m + 1) * 128],
                                     rhs=sT3[:, k, sl],
                                     start=(k == 0), stop=(k == KC - 1))
                sig = ad.tile([128, 512], bf16, name="sig", tag="sig")
                nc.scalar.activation(out=sig, in_=p1[:, 0:512], func=AF.Sigmoid,
                                     bias=bs_sb[:, m:m + 1], scale=1.0)
                p2t = ps_big.tile([128, 512], f32, name="psb", tag="psb")
                for k in range(KC):
                    nc.tensor.matmul(p2t[:, 0:512], lhsT=wns_sb[:, k, m * 128:(m + 1) * 128],
                                     rhs=sT3[:, k, sl],
                                     start=(k == 0), stop=(k == KC - 1))
                nc.gpsimd.tensor_mul(out=aT3[:, m, sl], in0=sig, in1=lnaT3[:, m, sl])
                nc.vector.tensor_add(out=aT3[:, m, sl], in0=aT3[:, m, sl], in1=p2t[:, 0:512])
            return run
        for m in range(KC):
            for half in range(2):
                b_tasks.append(adaln_task(m, half))

        def kT_task(m, half):
            def run():
                sl = slice(half * 512, (half + 1) * 512)
                pk = ps_big.tile([128, 512], f32, name="psb", tag="psb")
                for k in range(KC):
                    nc.tensor.matmul(pk[:, 0:512], lhsT=wk_sb[:, k, m * 128:(m + 1) * 128],
                                     rhs=aT3[:, k, sl],
                                     start=(k == 0), stop=(k == KC - 1))
                copy2(kT[m][:, sl], pk[:, 0:512])
            return run
        for m in range(MC4):
            for half in range(2):
                b_tasks.append(kT_task(m, half))

        def qg_task(m):
            def run():
                pq = psmf()[:, 0:128]
                for k in range(KC):
                    nc.tensor.matmul(pq, lhsT=wq_sb[:, k, m * 128:(m + 1) * 128],
                                     rhs=aT3[:, k, 0:IB],
                                     start=(k == 0), stop=(k == KC - 1))
                nc.scalar.activation(out=qT[m], in_=pq, func=AF.Identity,
                                     bias=bq_sb[:, m:m + 1], scale=1.0)
                pg = psmf()[:, 0:128]
                for k in range(KC):
                    nc.tensor.matmul(pg, lhsT=wg_sb[:, k, m * 128:(m + 1) * 128],
                                     rhs=aT3[:, k, 0:IB],
                                     start=(k == 0), stop=(k == KC - 1))
                nc.scalar.activation(out=gT[m], in_=pg, func=AF.Sigmoid)
            return run
        for m in range(MC4):
            b_tasks.append(qg_task(m))

        def v_task(j):
            def run():
                pv = ps_big.tile([128, 512], f32, name="psb", tag="psb")
                for k in range(KC):
                    nc.tensor.matmul(pv[:, 0:HDP], lhsT=aT3[:, k, j * 128:(j + 1) * 128],
                                     rhs=wv_sb[:, k, :],
                                     start=(k == 0), stop=False)
                # +1.0 at every pad slot of each head (vpad one-hot row): the
                # AV matmul then emits the softmax denominator at psum rows
                # 32h+24..31 for free
                nc.tensor.matmul(pv[:, 0:HDP], lhsT=ones1[:, :],
                                 rhs=vpad_sb[:, :], start=False, stop=True,
                                 tile_position=(0, 0))
                copy2(V[j], pv[:, 0:HDP])
            return run
        for j in range(NJC):
            b_tasks.append(v_task(j))

        # =================================================================
        # Phase C: z stream, interleaved with phase B tasks
        # =================================================================
        zp = bc_ctx.enter_context(tc.tile_pool(name="zp", bufs=int(os.environ.get("KERNEL_ZPB", "7"))))
        zpb = bc_ctx.enter_context(tc.tile_pool(name="zpb", bufs=int(os.environ.get("KERNEL_ZPBB", "1"))))
        stg = bc_ctx.enter_context(tc.tile_pool(name="stg", bufs=int(os.environ.get("KERNEL_STGB", "4"))))
        xs_ctx = ExitStack()
        xp = xs_ctx.enter_context(tc.tile_pool(name="xp", bufs=1))
        xs.extend(xp.tile([128, C_S], f32, name=f"x{i}", tag=f"x{i}")
                  for i in range(int(os.environ.get("KERNEL_XB", "16"))))
        SGT = int(os.environ.get("KERNEL_SGT", "2"))  # chunks per staging tile
        _sq = [0]
        _stage = [None]
        _zi = [0, 0]                              # fp8 / bf16 chunk counters

        _stage_t0 = [0]

        def emit_chunk(t, e):
            # e = emission index; stage groups batch 8 same-parity chunks
            q = e % SGT
            if q == 0:
                _stage[0] = stg.tile([128, SGT * 512], bf16, name="stage", tag="stage")
                _stage_t0[0] = t
            is_b = _chunk_is_bf16(t)
            dma_c = dma_z
            if is_b:
                tb = IBPOS[t]
                zz = zpb.tile([128, 2, ZCH], bf16, name="zzb", tag="zzb")
                dma_c(out=zz[:, 0, :], in_=p["z_tb"][:, tb * ZCH:(tb + 1) * ZCH])
                # bf16 square on DVE hits the 2x perf mode
                nc.vector.tensor_mul(out=zz[:, 1, :], in0=zz[:, 0, :], in1=zz[:, 0, :])
            else:
                t8 = I8POS[t]
                zz = zp.tile([128, 2, ZCH], ZDT, name="zz", tag="zz")
                dma_c(out=zz[:, 0, :], in_=p["z_t"][:, t8 * ZCH:(t8 + 1) * ZCH])
                # fp8 squares rotate Pool/DVE/ACT (Square is in every ACT
                # table, so no table-load cost)
                _sqpat = os.environ.get("KERNEL_SQPAT", "PPD")
                eng = _sqpat[_sq[0] % len(_sqpat)]
                _sq[0] += 1
                if eng == "P":
                    nc.gpsimd.tensor_mul(out=zz[:, 1, :], in0=zz[:, 0, :], in1=zz[:, 0, :])
                elif eng == "A":
                    nc.scalar.activation(out=zz[:, 1, :], in_=zz[:, 0, :],
                                         func=AF.Square)
                else:
                    nc.vector.tensor_mul(out=zz[:, 1, :], in0=zz[:, 0, :], in1=zz[:, 0, :])
            pstat = ps_stat.tile([128, 512], f32, name="pstat", tag="pstat")
            for s in range(4):
                sl = slice(s * 512, (s + 1) * 512)
                if not is_b and USE_Z8:
                    # DoubleRow disallows PE column tiling; quarter s's stats
                    # land at psum rows 32s via a block-shifted weight copy.
                    nc.tensor.matmul(pstat[:, :], lhsT=wb2_sb[:, s, :, :],
                                     rhs=zz[:, :, sl], start=(s == 0), stop=(s == 3),
                                     perf_mode=mybir.MatmulPerfMode.DoubleRow)
                else:
                    nc.tensor.matmul(pstat[32 * s:32 * s + 32, :], lhsT=wb2b_sb[:, 0, :],
                                     rhs=zz[:, 0, sl], start=True, stop=False,
                                     tile_position=(0, 32 * s))
                    nc.tensor.matmul(pstat[32 * s:32 * s + 32, :], lhsT=wb2b_sb[:, 1, :],
                                     rhs=zz[:, 1, sl], start=False, stop=True,
                                     tile_position=(0, 32 * s))
            copy2(_stage[0][:, q * 512:(q + 1) * 512], pstat)
            if q == SGT - 1:
                t0 = _stage_t0[0]
                b = braw_h[t0 % 2]
                base = (t0 // 2) * 512
                # odd-half stage writes alternate ACT/SP (drain fast at the
                # end: rs1 waits on the last of them)
                if t0 % 2 == 0:
                    qw = dma_z
                else:
                    qw = dma_w if (t0 // 2) % 2 == 0 else dma_z
                qw(out=b[:, base:base + SGT * 512], in_=_stage[0])

        # even-j chunks first (with phase B woven in); odd-j chunks then
        # overlap with the half-0 attention passes.
        if os.environ.get("KERNEL_SPLIT", "1") == "1":
            evens = list(range(0, NZC, 2))
            odds = list(range(1, NZC, 2))
        else:
            evens = list(range(NZC))
            odds = []
        bi_ = 0
        for n_, t in enumerate(evens):
            emit_chunk(t, n_)
            while bi_ * len(evens) < (n_ + 1) * len(b_tasks):
                b_tasks[bi_]()
                bi_ += 1
        while bi_ < len(b_tasks):
            b_tasks[bi_]()
            bi_ += 1
        xs_ctx.close()
        if os.environ.get("KERNEL_NOBAR", "1") != "1":
            tc.strict_bb_all_engine_barrier()

        # =================================================================
        # Phase D/E half 0 (j < 512), overlapped with the odd z chunks
        # =================================================================
        def row_view(r, jh):
            # [i, j-half] plane of stats row r: partition p=i reads braw_h[jh]
            # row 32*(i//32)+r, cols (i%32)*512 + j  (DRAM: flat)
            base = braw_h[jh][:, :]
            return bass.AP(
                tensor=base.tensor,
                offset=base.offset + r * IJ4H,
                ap=[[32 * IJ4H, 4], [512, 32], [1, 512]],
            )

        ZINV = 1.0 / (C_Z * WBSCALE)

        def d_half(jh):
            # jh=0 runs while SP is busy (ACT queue idle); jh=1 the reverse
            q1, q2 = (dma_w, dma_w) if jh == 0 else (dma_z, nc.gpsimd.dma_start)
            Sh = p2.tile([IB, 512], bf16, name=f"S{jh}", tag=f"S{jh}")
            q1(out=Sh, in_=row_view(16, jh))
            Qh = p2.tile([IB, 512], bf16, name=f"Qh{jh}", tag=f"Qh{jh}")
            q2(out=Qh, in_=row_view(17, jh))
            msq = att.tile([IB, 512], f32, name="msq", tag="msq")
            nc.vector.scalar_tensor_tensor(out=msq, in0=Sh, scalar=ZINV * ZINV,
                                           in1=Sh, op0=ALU.mult, op1=ALU.mult)
            var = att.tile([IB, 512], f32, name="var", tag="var")
            nc.vector.scalar_tensor_tensor(out=var, in0=Qh, scalar=ZINV,
                                           in1=msq, op0=ALU.mult, op1=ALU.subtract)
            # rs' = exp(-0.5*ln(var+eps) - ln WBSCALE)
            lnv2 = att.tile([IB, 512], f32, name="lnv2", tag="lnv2")
            nc.scalar.activation(out=lnv2, in_=var, func=AF.Ln, bias=eps_t)
            rsh = p2.tile([IB, 512], bf16, name=f"rs{jh}", tag=f"rs{jh}")
            nc.scalar.activation(out=rsh, in_=lnv2, func=AF.Exp, scale=-0.5,
                                 bias=nlb_t)
            return rsh

        rs0 = d_half(0)
        bh0 = []
        for h in range(H):
            b = p2.tile([IB, 512], bf16, name=f"bh0_{h}", tag=f"bh0_{h}")
            (dma_p, dma_w)[h % 2](out=b, in_=row_view(h, 0))
            bh0.append(b)
        e0s = [p2.tile([128, 512], bf16, name=f"e0_{h}", tag=f"e0_{h}")
               for h in range(H)]
        # all 4 head-groups' AV accumulators in ONE psum bank per half:
        # group m at cols [128m, 128m+128); rows 32k+24..31 = denominators
        ops0 = ps_o.tile([128, 512], f32, name="ops0", tag="ops0")
        ops1 = ps_o.tile([128, 512], f32, name="ops1", tag="ops1")

        def logits_T(h, jh, X, Eh):
            # E^T[jb*128+jj, ii] = exp(qk+bias)[i, 512jh+128jb+jj]
            c4, r = h // 4, 32 * (h % 4)
            Lp = ps_big.tile([128, 512], f32, name="psb", tag="psb")
            for jb in range(4):
                sl = slice(jb * 128, (jb + 1) * 128)
                j0 = 512 * jh + 128 * jb
                nc.tensor.matmul(Lp[:, sl], lhsT=kT[c4][r:r + DP, j0:j0 + 128],
                                 rhs=qT[c4][r:r + DP, 0:IB],
                                 start=True, stop=False, tile_position=(r, 0))
                nc.tensor.matmul(Lp[:, sl], lhsT=X[:, sl], rhs=ident[:, 0:IB],
                                 start=False, stop=True, tile_position=(0, 0))
            nc.scalar.activation(out=Eh, in_=Lp[:, :], func=AF.Exp)

        def av_half(h, jh, Eh, opst):
            c4, r = h // 4, 32 * (h % 4)
            for k4 in range(4):
                jc = jh * 4 + k4
                nc.tensor.matmul(opst[r:r + DP, c4 * 128:c4 * 128 + IB],
                                 lhsT=V[jc][:, DP * h:DP * h + DP],
                                 rhs=Eh[:, k4 * 128:(k4 + 1) * 128],
                                 start=(k4 == 0), stop=(k4 == 3),
                                 tile_position=(0, r))

        def half0_head(h):
            def run():
                X = att.tile([IB, 512], bf16, name="X", tag="X")
                nc.gpsimd.tensor_mul(out=X, in0=bh0[h], in1=rs0)
                logits_T(h, 0, X, e0s[h])
                av_half(h, 0, e0s[h], ops0)
            return run

        h0_tasks = [half0_head(h) for h in range(H)]
        hi = 0
        DL = int(os.environ.get("KERNEL_DL", "4"))  # delay before first half-0 head
        for n_, t in enumerate(odds):
            emit_chunk(t, len(evens) + n_)
            if n_ >= DL:
                while hi * (len(odds) - DL) < (n_ - DL + 1) * len(h0_tasks):
                    h0_tasks[hi]()
                    hi += 1
        while hi < len(h0_tasks):
            h0_tasks[hi]()
            hi += 1
        bc_ctx.close()
        if os.environ.get("KERNEL_NOBAR", "1") != "1":
            tc.strict_bb_all_engine_barrier()

        # PE p-state warm-keeper: the cost model halves matmul speed after
        # any idle gap (3us ramp). Dummy matmuls bridge the rs1 stall and
        # the per-head exp waits so real tail matmuls run at full clock.
        dum_ps = ps_stat.tile([128, 512], f32, name="pstat", tag="pstat")

        def pe_warm(n):
            for _ in range(n):
                nc.tensor.matmul(dum_ps[0:128, 0:128], lhsT=ident[:, 0:128],
                                 rhs=ident[:, 0:128], start=True, stop=True)
        pe_warm(int(os.environ.get("KERNEL_WARM0", "100")))

        # =================================================================
        # Half 1 (j >= 512) + late normalization tail
        # =================================================================
        rs1 = d_half(1)
        # prefetch all 16 half-1 bias rows into the (dead) bh0 slots --
        # per-head pooled gathers would otherwise serialize the head loop
        # behind a full DMA round-trip every other head
        bh1s = []
        for h in range(H):
            b = p2.tile([IB, 512], bf16, name=f"bh0_{h}", tag=f"bh0_{h}")
            (dma_z, nc.gpsimd.dma_start, dma_w)[h % 3](out=b, in_=row_view(h, 1))
            bh1s.append(b)
        # ops0 is final once the half-0 heads are done: stage its psum->sbuf
        # copies here so only the add sits on each group's terminal chain
        S0s = [p2.tile([128, IB], f32, name=f"S0_{m}", tag=f"S0_{m}")
               for m in range(MC4)]
        for m in range(MC4):
            nc.scalar.activation(out=S0s[m], in_=ops0[:, m * 128:m * 128 + IB],
                                 func=AF.Copy)
        _WARMH = int(os.environ.get("KERNEL_WARMH", "3"))
        _WARMA = int(os.environ.get("KERNEL_WARMA", "2"))
        og = [p2.tile([128, IB], bf16, name=f"og{m}", tag=f"og{m}") for m in range(MC4)]
        for h in range(H):
            c4, r = h // 4, 32 * (h % 4)
            X = att.tile([IB, 512], bf16, name="X", tag="X")
            nc.gpsimd.tensor_mul(out=X, in0=bh1s[h], in1=rs1)
            E1 = att.tile([128, 512], bf16, name="E1", tag="E1")
            logits_T(h, 1, X, E1)
            pe_warm(_WARMH)
            av_half(h, 1, E1, ops1)
            pe_warm(_WARMA)
            if h % 4 == 3:
                # o = ops0+ops1; every pad row holds the head's denominator;
                # reciprocal the whole block (finite everywhere), then a
                # selector matmul broadcasts row 32k+24 onto rows 32k+d
                cs = slice(c4 * 128, c4 * 128 + IB)
                S = att.tile([128, IB], f32, name="Ssum", tag="Ssum")
                nc.vector.tensor_add(out=S, in0=ops1[:, cs], in1=S0s[c4])
                rfull = att.tile([128, IB], f32, name="rfull", tag="rfull")
                nc.vector.reciprocal(out=rfull, in_=S)
                rexp = psmf()[0:128, 0:IB]
                nc.tensor.matmul(rexp, lhsT=w4_sb[:, :], rhs=rfull[:, :],
                                 start=True, stop=True)
                nc.vector.tensor_mul(out=og[c4], in0=S, in1=gT[c4])
                nc.vector.tensor_mul(out=og[c4], in0=og[c4], in1=rexp)

        # =================================================================
        # Phase F: output projection + final gate
        # =================================================================
        pout = ps_big.tile([128, 512], f32, name="psb", tag="psb")
        for m in range(MC4):
            nc.tensor.matmul(pout[0:IB, 0:C_S], lhsT=og[m], rhs=wo_sb[:, m, :],
                             start=(m == 0), stop=(m == MC4 - 1))
        fin = p2.tile([IB, C_S], f32, name="fin", tag="fin")
        nc.vector.tensor_mul(out=fin, in0=pout[0:IB, 0:C_S], in1=sg)
        dma_z(out=p["out"][:, :], in_=fin)


def _prep_host(inputs):
    """Fold weights, pad heads, shard + rotate per core."""
    import ml_dtypes
    i = {k: np.asarray(v, dtype=np.float32) for k, v in inputs.items()}
    lnsw = i["adaln_lns_w"]                      # [C_S]
    w_ws = np.ascontiguousarray(lnsw[:, None] * i["adaln_ws"]).astype(ml_dtypes.bfloat16)
    w_wns = np.ascontiguousarray(lnsw[:, None] * i["adaln_wns"]).astype(ml_dtypes.bfloat16)

    def pad_heads(w, scale=1.0):                 # [C_S, H*D] -> [C_S, H*DP]
        wp = np.zeros((C_S, HDP), np.float32)
        for h in range(H):
            wp[:, h * DP:h * DP + D] = w[:, h * D:(h + 1) * D] * scale
        return wp.astype(ml_dtypes.bfloat16)

    w_q = pad_heads(i["wq"], SCALE)
    b_q = np.zeros((HDP,), np.float32)
    for h in range(H):
        b_q[h * DP:h * DP + D] = i["bq"][h * D:(h + 1) * D] * SCALE
    w_k = pad_heads(i["wk"])
    w_v = pad_heads(i["wv"])
    w_g = pad_heads(i["wg"])
    w_o = np.zeros((HDP, C_S), np.float32)
    for h in range(H):
        w_o[h * DP:h * DP + D, :] = i["wo"][h * D:(h + 1) * D, :]
    w_o = w_o.astype(ml_dtypes.bfloat16)

    zdt = ml_dtypes.float8_e4m3 if USE_Z8 else ml_dtypes.bfloat16
    wbp = i["lnb_w"][:, None] * i["wb"]          # [C_Z, H]
    wbc = (wbp - wbp.mean(axis=0, keepdims=True)) * WBSCALE
    wb_aug = np.zeros((C_Z, 32), np.float32)
    wb_aug[:, :H] = wbc
    wb_aug[:, 16] = WBSCALE                      # sum z   (scaled)
    sq_aug = np.zeros((C_Z, 32), np.float32)
    sq_aug[:, 17] = WBSCALE                      # sum z^2 (scaled)
    if USE_Z8:
        wb2 = np.zeros((C_Z, 4, 2, 128), np.float32)
        for s in range(4):
            wb2[:, s, 0, 32 * s:32 * s + 32] = wb_aug
            wb2[:, s, 1, 32 * s:32 * s + 32] = sq_aug
    else:
        wb2 = np.stack([wb_aug, sq_aug], axis=1)  # [C_Z, 2, 32]
    wb2 = wb2.astype(zdt)
    wb2b = np.stack([wb_aug, sq_aug], axis=1).astype(ml_dtypes.bfloat16)
    w4exp = np.zeros((128, 128), np.float32)
    for k in range(4):
        w4exp[32 * k + D, 32 * k:32 * k + D] = 1.0
    vpad = np.zeros((1, HDP), np.float32)
    for h in range(H):
        vpad[0, h * DP + D:(h + 1) * DP] = 1.0
    vpad = vpad.astype(ml_dtypes.bfloat16)

    i8 = [t for t in range(NZC) if not _chunk_is_bf16(t)]
    ib_ = [t for t in range(NZC) if _chunk_is_bf16(t)]

    z0 = i["z_ij"][0]                            # [N, N, C_Z]
    zT_full = np.ascontiguousarray(z0.transpose(2, 0, 1))  # [C_Z, N(i), N(j)]

    in_maps = []
    for c in range(NCORES):
        i0 = c * IB
        ridx = (np.arange(N) + i0) % N           # token rotation
        zc = zT_full[:, i0:i0 + IB, :][:, :, ridx]          # [C_Z, IB, N]
        zarr = zc.reshape(C_Z, 4, IJ // (4 * 512), 512).transpose(0, 2, 1, 3)
        zarr = np.ascontiguousarray(zarr.reshape(C_Z, NZC, ZCH))
        z8a = np.ascontiguousarray(
            zarr[:, i8, :].reshape(C_Z, max(NZ8, 1) * ZCH)).astype(zdt)
        zba = np.ascontiguousarray(
            zarr[:, ib_, :].reshape(C_Z, -1) if NZB else
            np.zeros((C_Z, ZCH), np.float32)).astype(ml_dtypes.bfloat16)
        in_maps.append({
            "z_t": z8a, "z_tb": zba, "wb2b": wb2b,
            "a_in": np.ascontiguousarray(i["a_i"][0][ridx]),
            "s_in": np.ascontiguousarray(i["s_i"][0][ridx]),
            "w_ws": w_ws, "w_wns": w_wns, "b_s": i["adaln_bs"],
            "w_q": w_q, "b_q": b_q, "w_k": w_k, "w_v": w_v, "w_g": w_g,
            "wb2": wb2,
            "w_o": w_o, "w_sg": i["ws"].astype(ml_dtypes.bfloat16), "b_sg": i["bs"],
            "w4exp": w4exp, "vpad": vpad,
        })
    return in_maps


LAST_EXEC_NS = None


def _run_timed(nc, in_maps, n_iters=6):
    """Execute via PJRT with device-resident inputs; time repeated calls."""
    import time as _time

    import jax
    from jax.sharding import Mesh, PartitionSpec
    from jax.experimental.shard_map import shard_map
    from concourse import mybir as _mb
    from concourse.bass2jax import (_bass_exec_p, install_neuronx_cc_hook,
                                    partition_id_tensor)

    install_neuronx_cc_hook()
    n_cores = len(in_maps)
    pname = nc.partition_id_tensor.name if nc.partition_id_tensor else None

    in_names, out_names, out_avals, zero_outs = [], [], [], []
    for alloc in nc.m.functions[0].allocations:
        if not isinstance(alloc, _mb.MemoryLocationSet):
            continue
        name = alloc.memorylocations[0].name
        if alloc.kind == "ExternalInput":
            if name != pname:
                in_names.append(name)
        elif alloc.kind == "ExternalOutput":
            out_names.append(name)
            shape = tuple(alloc.tensor_shape)
            dtype = _mb.dt.np(alloc.dtype)
            out_avals.append(jax.core.ShapedArray(shape, dtype))
            zero_outs.append(np.zeros(shape, dtype))
    n_params = len(in_names)
    all_in_names = in_names + out_names
    if pname is not None:
        all_in_names = all_in_names + [pname]

    def _body(*args):
        operands = list(args)
        if pname is not None:
            operands.append(partition_id_tensor())
        outs = _bass_exec_p.bind(
            *operands,
            out_avals=tuple(out_avals),
            in_names=tuple(all_in_names),
            out_names=tuple(out_names),
            lowering_input_output_aliases=(),
            sim_require_finite=True,
            sim_require_nnan=True,
            nc=nc,
        )
        return tuple(outs)

    devices = jax.devices()[:n_cores]
    mesh = Mesh(np.asarray(devices), ("core",))
    in_specs = (PartitionSpec("core"),) * (n_params + len(out_names))
    out_specs = (PartitionSpec("core"),) * len(out_names)
    fn = jax.jit(shard_map(_body, mesh=mesh, in_specs=in_specs,
                           out_specs=out_specs, check_rep=False),
                 keep_unused=True)

    concat_in = [
        np.concatenate([np.asarray(in_maps[c][nm]) for c in range(n_cores)], axis=0)
        for nm in in_names
    ]
    concat_zeros = [
        np.zeros((n_cores * z.shape[0], *z.shape[1:]), z.dtype) for z in zero_outs
    ]
    sharding = jax.sharding.NamedSharding(mesh, PartitionSpec("core"))
    dev_in = [jax.device_put(a, sharding) for a in concat_in]
    dev_zero = [jax.device_put(a, sharding) for a in concat_zeros]

    out_arrs = fn(*dev_in, *dev_zero)      # warmup + compile
    jax.block_until_ready(out_arrs)
    best = float("inf")
    for _ in range(n_iters):
        t0 = _time.perf_counter()
        r = fn(*dev_in, *dev_zero)
        jax.block_until_ready(r)
        best = min(best, _time.perf_counter() - t0)
    out_arrs = r
    results = [
        {nm: np.asarray(out_arrs[i]).reshape(n_cores, *out_avals[i].shape)[c]
         for i, nm in enumerate(out_names)}
        for c in range(n_cores)
    ]
    return results, best


def kernel(**inputs) -> np.ndarray:
    global LAST_EXEC_NS
    if "nc" not in _CACHED:
        _CACHED["nc"] = _build_program()
    nc = _CACHED["nc"]
    in_maps = _prep_host(inputs)
    if os.environ.get("KERNEL_TIMED", "0") == "1":
        outs, best_s = _run_timed(nc, in_maps)
        LAST_EXEC_NS = int(best_s * 1e9)
    else:
        kw = {}
        if os.environ.get("KERNEL_TRACE", "0") == "1":
            kw = dict(trace=True, tmpdir="/tmp/kern_trace")
            os.makedirs("/tmp/kern_trace", exist_ok=True)
        res = run_bass_kernel_spmd(nc, in_maps, list(range(NCORES)), **kw)
        LAST_EXEC_NS = getattr(res, "exec_time_ns", None)
        outs = res.results
    full = np.concatenate([outs[c]["out"] for c in range(NCORES)], axis=0)
    return full[None, :, :].astype(np.float32)

